# revision 7
# baseline (speedup 1.0000x reference)
"""nn_LlamaAttention kernel: tensor-parallel over heads across 8 NeuronCores.

Core r owns heads 4r..4r+3 (output-channel slice [512r, 512r+512) of
wq/wk/wv and the matching KV-cache slice).  Device program per core:

  P1  activation quant (int4 groups of 128 + int8 outlier group) in
      token-major layout, PE-transposed to channel-major hq (f16).
  P2  Q/K/V projections (f16 matmuls, fp32 PSUM), int4 KV quant
      (groups of 64) and RoPE in token-major layout.
  P3  ragged causal prefill attention per (seq, head): scores^T blocks
      [Lk,Lq] on PE, Exp on ACT, causal mask multiply, softmax sums via
      ones-matmul, PV matmul, normalize; plus batched decode against the
      f16 KV cache.
  P4  AllToAll (head-sharded -> token-sharded), indirect row-gather of
      the channel permutation, o-quant (PE-transpose absmax + ones-
      matmul broadcast), row-parallel o_proj with full wo (f16).

Host only shards/preps inputs (transposes, f16 casts, trig tables) and
concatenates the 8 token-block outputs.  Falls back to an exact fp32
numpy implementation if the device path fails.
"""
import math
import os
import sys
import time
import traceback

import numpy as np

sys.path.insert(0, "/opt/trn_rl_repo")

H = 4096
NH = 32
HD = 128
GROUP = 128
N_OUTLIER = 128
KV_GROUP = 64
PREFILL = (1536, 1024, 768, 512)
SEQS = ((0, 1536), (1536, 1024), (2560, 768), (3328, 512))
DOFF = sum(PREFILL)
DEC_B = 16
PAST = 512
T = DOFF + DEC_B
SCALE = 1.0 / math.sqrt(HD)
NCORES = 8
HPC = NH // NCORES          # heads per core
OC = HPC * HD               # 512 out channels per core
TB = T // NCORES            # 482 tokens per core for o_proj
NG = H // GROUP             # 32 activation-quant groups

LAST_HW_EXEC_NS = None


# ==========================================================================
# walrus workaround: this toolchain accepts at most ONE SyncWait per
# instruction.  After Tile lowering, redistribute multi-wait sync_infos
# onto single-wait NoOp carriers inserted just before each instruction.
# ==========================================================================
def _apply_tilepatch(tile_mod, mybir):
    if getattr(tile_mod.TileContext, "_waitsplit_patched", False):
        return
    counter = [0]

    def split_all_waits(nc):
        for f in nc.m.functions:
            for bb in f.blocks:
                insts = bb.instructions
                idx = 0
                while idx < len(insts):
                    inst = insts[idx]
                    si = inst.sync_info
                    waits = list(si.on_wait) if si is not None else []
                    if len(waits) > 1:
                        upds = list(si.on_update) if si is not None else []
                        carriers = []
                        for w in waits[:-1]:
                            counter[0] += 1
                            nop = mybir.InstNoOp(
                                name=f"I-waitsplit-{counter[0]}",
                                sync_info=mybir.SyncInfo(
                                    on_wait=[w], on_update=[]),
                                bass_nofuse=True,
                                engine=inst.engine,
                            )
                            carriers.append(nop)
                        inst.sync_info = mybir.SyncInfo(
                            on_wait=waits[-1:], on_update=upds)
                        for j, nop in enumerate(carriers):
                            insts.insert(idx + j, nop)
                            nc.register_instruction(nop)
                        idx += len(carriers)
                    idx += 1

    orig_exit = tile_mod.TileContext.__exit__

    def patched_exit(self, *a, **k):
        r = orig_exit(self, *a, **k)
        split_all_waits(self.nc)
        return r

    tile_mod.TileContext.__exit__ = patched_exit
    tile_mod.TileContext._waitsplit_patched = True


# ==========================================================================
# host prep
# ==========================================================================
def _rope_tables():
    inv = 1.0 / (10000.0 ** (np.arange(0, HD, 2, dtype=np.float32) / HD))
    pos = np.zeros(T, dtype=np.float32)
    for t0, L in SEQS:
        pos[t0:t0 + L] = np.arange(L, dtype=np.float32)
    pos[DOFF:] = float(PAST)
    ang = pos[:, None] * inv[None, :]          # [T, 64]
    return np.cos(ang).astype(np.float32), np.sin(ang).astype(np.float32)


def _diag_masks():
    # dmask[lk, m, lq] = 1 if lq >= 128*m + lk else 0   (within a 512 chunk)
    lk = np.arange(128)[:, None, None]
    m = np.arange(4)[None, :, None]
    lq = np.arange(512)[None, None, :]
    return (lq >= 128 * m + lk).astype(np.float16)


def _host_prep(hidden, wq, wk, wv, wo, ridx, cache_k, cache_v):
    cos_t, sin_t = _rope_tables()
    dmask = _diag_masks()
    wo_t = np.ascontiguousarray(wo.T).astype(np.float16)
    ridx_sb = np.ascontiguousarray(
        ridx.astype(np.int32).reshape(NG, GROUP).T)          # [128, 32]
    in_maps = []
    for r in range(NCORES):
        sl = slice(OC * r, OC * (r + 1))
        hsl = slice(HPC * r, HPC * (r + 1))
        kct = np.ascontiguousarray(
            cache_k[:, hsl].transpose(1, 0, 3, 2)).astype(np.float16)
        vct = np.ascontiguousarray(
            cache_v[:, hsl].transpose(0, 1, 2, 3)).astype(np.float16)
        in_maps.append({
            "hidden": hidden,
            "wq_t": np.ascontiguousarray(wq[sl].T).astype(np.float16),
            "wk_t": np.ascontiguousarray(wk[sl].T).astype(np.float16),
            "wv_t": np.ascontiguousarray(wv[sl].T).astype(np.float16),
            "wo_t": wo_t,
            "cos_t": cos_t,
            "sin_t": sin_t,
            "kct": kct,                  # [HPC, DEC_B, HD, PAST] f16 (cm)
            "vct": vct,                  # [DEC_B, HPC, PAST, HD] f16 (tm)
            "ridx_sb": ridx_sb,
            "dmask": dmask,
        })
    return in_maps


def _tok_runs(g0, n):
    """Split global-token range [g0, g0+n) at TB boundaries.
    Returns list of (tb, ti, off, ln)."""
    runs = []
    off = 0
    while off < n:
        t = g0 + off
        tb, ti = divmod(t, TB)
        ln = min(n - off, TB - ti)
        runs.append((tb, ti, off, ln))
        off += ln
    return runs


# ==========================================================================
# device program
# ==========================================================================
def _build_program():
    import concourse.bass as bass
    import concourse.tile as tile
    from concourse import mybir
    from concourse.masks import make_identity

    _apply_tilepatch(tile, mybir)

    F32, F16, I32 = mybir.dt.float32, mybir.dt.float16, mybir.dt.int32
    AF = mybir.ActivationFunctionType
    OP = mybir.AluOpType
    AX = mybir.AxisListType

    nc = bass.Bass(num_devices=NCORES)

    def P(name, shape, dt, out=False):
        return nc.declare_dram_parameter(name, list(shape), dt, isOutput=out)

    hidden = P("hidden", [T, H], F32)
    wq_t = P("wq_t", [H, OC], F16)
    wk_t = P("wk_t", [H, OC], F16)
    wv_t = P("wv_t", [H, OC], F16)
    wo_t = P("wo_t", [H, H], F16)
    cos_t = P("cos_t", [T, 64], F32)
    sin_t = P("sin_t", [T, 64], F32)
    kct = P("kct", [HPC, DEC_B, HD, PAST], F16)
    vct = P("vct", [DEC_B, HPC, PAST, HD], F16)
    ridx_sb = P("ridx_sb", [128, NG], I32)
    dmask = P("dmask", [128, 4, 512], F16)
    out = P("out", [H, TB], F32, out=True)

    # DRAM intermediates
    hq_cm = nc.dram_tensor("hq_cm", [NG, 128, T], F16)
    q_tm = nc.dram_tensor("q_tm", [T, OC], F16)
    k_tm = nc.dram_tensor("k_tm", [T, OC], F16)
    v_tm = nc.dram_tensor("v_tm", [T, OC], F16)
    a2a_in = nc.dram_tensor("a2a_in", [NCORES, OC, TB], F16)
    a2a_out = nc.dram_tensor("a2a_out", [H, TB], F16)

    rg = list(range(NCORES))

    with tile.TileContext(nc) as tc:
        # ------------------------------------------------------- residents
        res = tc.alloc_tile_pool(name="res", bufs=1)
        ident16 = res.tile([128, 128], F16)
        make_identity(nc, ident16[:])
        ident32 = res.tile([128, 128], F32)
        make_identity(nc, ident32[:])
        ones16 = res.tile([128, 1], F16)
        nc.vector.memset(ones16[:], 1.0)
        ones32 = res.tile([1, 128], F32)
        nc.vector.memset(ones32[:], 1.0)
        dm = res.tile([128, 4, 512], F16)
        nc.sync.dma_start(out=dm[:], in_=dmask[:])
        rixt = res.tile([128, NG], I32)
        nc.sync.dma_start(out=rixt[:], in_=ridx_sb[:])

        # =========================================================== P1: hq
        with tc.tile_pool(name="p1", bufs=2) as p1, \
             tc.tile_pool(name="p1s", bufs=2) as p1s, \
             tc.tile_pool(name="p1ps", bufs=4, space="PSUM") as p1ps:
            n_tiles = (T + 127) // 128
            for tb in range(n_tiles):
                t0 = 128 * tb
                rows = min(128, T - t0)
                x = p1.tile([128, H], F32, tag="x")
                nc.sync.dma_start(out=x[:rows], in_=hidden[t0:t0 + rows])
                am = p1s.tile([128, NG], F32, tag="am")
                nc.vector.tensor_reduce(
                    out=am[:rows], in_=x[:rows].rearrange(
                        "p (g d) -> p g d", g=NG),
                    axis=AX.X, op=OP.max, apply_absolute_value=True)
                sc = p1s.tile([128, NG], F32, tag="sc")
                nc.vector.tensor_scalar(
                    out=sc[:rows, :NG - 1], in0=am[:rows, :NG - 1],
                    scalar1=1.0 / 7.0, scalar2=1e-6, op0=OP.mult, op1=OP.add)
                nc.vector.tensor_scalar(
                    out=sc[:rows, NG - 1:], in0=am[:rows, NG - 1:],
                    scalar1=1.0 / 127.0, scalar2=1e-6, op0=OP.mult, op1=OP.add)
                rs = p1s.tile([128, NG], F32, tag="rs")
                nc.vector.reciprocal(rs[:rows], sc[:rows])
                qf = p1.tile([128, H], F32, tag="qf")
                for g in range(NG):
                    gs = slice(g * GROUP, (g + 1) * GROUP)
                    nc.vector.tensor_scalar_mul(
                        qf[:rows, gs], x[:rows, gs], rs[:rows, g:g + 1])
                qi = p1.tile([128, H], I32, tag="qi")
                nc.vector.tensor_copy(qi[:rows], qf[:rows])
                qh = p1.tile([128, H], F16, tag="qh")
                for g in range(NG):
                    gs = slice(g * GROUP, (g + 1) * GROUP)
                    nc.gpsimd.tensor_scalar_mul(
                        qh[:rows, gs], qi[:rows, gs], sc[:rows, g:g + 1])
                tt = p1.tile([128, NG, 128], F16, tag="tt")
                for icb in range(NG):
                    gs = slice(icb * 128, (icb + 1) * 128)
                    pt = p1ps.tile([128, 128], F16, tag="pt")
                    nc.tensor.transpose(pt[:, :rows], qh[:rows, gs],
                                        ident16[:rows, :rows])
                    nc.scalar.copy(tt[:, icb, :rows], pt[:, :rows])
                nc.sync.dma_start(
                    out=hq_cm[:, :, t0:t0 + rows],
                    in_=tt[:, :, :rows].rearrange("p b r -> b p r"))

        # ========================================================= P2: QKV
        with tc.tile_pool(name="p2w", bufs=1) as p2w, \
             tc.tile_pool(name="p2", bufs=2) as p2, \
             tc.tile_pool(name="p2s", bufs=3) as p2s, \
             tc.tile_pool(name="p2ps", bufs=4, space="PSUM") as p2ps:
            wsb = {}
            for nm, wt in (("q", wq_t), ("k", wk_t), ("v", wv_t)):
                w = p2w.tile([128, NG, OC], F16, tag=f"w{nm}")
                nc.sync.dma_start(
                    out=w[:], in_=wt.ap().rearrange("(b p) o -> p b o", p=128))
                wsb[nm] = w

            chunks = [(i * 512, 512) for i in range(7)] + [(3584, 272)]
            for c0, cn in chunks:
                hq = p2.tile([128, NG, 512], F16, tag="hq")
                nc.sync.dma_start(
                    out=hq[:, :, :cn],
                    in_=hq_cm[:, :, c0:c0 + cn].rearrange("b p r -> p b r"))
                subs = [(o, min(128, cn - o)) for o in range(0, cn, 128)]
                for toff, rows in subs:
                    gt0 = c0 + toff
                    cosb = p2s.tile([128, 64], F32, tag="cosb")
                    sinb = p2s.tile([128, 64], F32, tag="sinb")
                    nc.sync.dma_start(out=cosb[:rows],
                                      in_=cos_t[gt0:gt0 + rows])
                    nc.sync.dma_start(out=sinb[:rows],
                                      in_=sin_t[gt0:gt0 + rows])
                    for nm in ("q", "k", "v"):
                        ps = p2ps.tile([128, OC], F32, tag="ps")
                        for icb in range(NG):
                            nc.tensor.matmul(
                                ps[:rows], hq[:, icb, toff:toff + rows],
                                wsb[nm][:, icb, :],
                                start=(icb == 0), stop=(icb == NG - 1))
                        sb = p2s.tile([128, OC], F32, tag="sb")
                        nc.scalar.copy(sb[:rows], ps[:rows])
                        if nm != "q":
                            # int4 KV quant, groups of 64
                            nkg = OC // KV_GROUP
                            kam = p2s.tile([128, nkg], F32, tag="kam")
                            nc.vector.tensor_reduce(
                                out=kam[:rows], in_=sb[:rows].rearrange(
                                    "p (g d) -> p g d", g=nkg),
                                axis=AX.X, op=OP.max,
                                apply_absolute_value=True)
                            ksc = p2s.tile([128, nkg], F32, tag="ksc")
                            nc.vector.tensor_scalar(
                                out=ksc[:rows], in0=kam[:rows],
                                scalar1=1.0 / 7.0, scalar2=1e-6,
                                op0=OP.mult, op1=OP.add)
                            krs = p2s.tile([128, nkg], F32, tag="krs")
                            nc.vector.reciprocal(krs[:rows], ksc[:rows])
                            kqf = p2s.tile([128, OC], F32, tag="kqf")
                            for g in range(nkg):
                                gs = slice(g * KV_GROUP, (g + 1) * KV_GROUP)
                                nc.vector.tensor_scalar_mul(
                                    kqf[:rows, gs], sb[:rows, gs],
                                    krs[:rows, g:g + 1])
                            kqi = p2s.tile([128, OC], I32, tag="kqi")
                            nc.vector.tensor_copy(kqi[:rows], kqf[:rows])
                            for g in range(nkg):
                                gs = slice(g * KV_GROUP, (g + 1) * KV_GROUP)
                                nc.gpsimd.tensor_scalar_mul(
                                    sb[:rows, gs], kqi[:rows, gs],
                                    ksc[:rows, g:g + 1])
                        if nm == "v":
                            vh = p2s.tile([128, OC], F16, tag="vh")
                            nc.vector.tensor_copy(vh[:rows], sb[:rows])
                            nc.sync.dma_start(out=v_tm[gt0:gt0 + rows],
                                              in_=vh[:rows])
                        else:
                            # RoPE (token-major), f16 out
                            rr = p2s.tile([128, OC], F16, tag="rr")
                            ta = p2s.tile([128, 64], F32, tag="ta")
                            tb_ = p2s.tile([128, 64], F32, tag="tb_")
                            for h in range(HPC):
                                lo = slice(h * 128, h * 128 + 64)
                                hi = slice(h * 128 + 64, h * 128 + 128)
                                nc.vector.tensor_tensor(
                                    out=ta[:rows], in0=sb[:rows, lo],
                                    in1=cosb[:rows], op=OP.mult)
                                nc.vector.tensor_tensor(
                                    out=tb_[:rows], in0=sb[:rows, hi],
                                    in1=sinb[:rows], op=OP.mult)
                                nc.vector.tensor_tensor(
                                    out=rr[:rows, lo], in0=ta[:rows],
                                    in1=tb_[:rows], op=OP.subtract)
                                nc.vector.tensor_tensor(
                                    out=ta[:rows], in0=sb[:rows, hi],
                                    in1=cosb[:rows], op=OP.mult)
                                nc.vector.tensor_tensor(
                                    out=tb_[:rows], in0=sb[:rows, lo],
                                    in1=sinb[:rows], op=OP.mult)
                                nc.vector.tensor_tensor(
                                    out=rr[:rows, hi], in0=ta[:rows],
                                    in1=tb_[:rows], op=OP.add)
                            dst = q_tm if nm == "q" else k_tm
                            nc.sync.dma_start(out=dst[gt0:gt0 + rows],
                                              in_=rr[:rows])

        # ==================================================== P3: prefill
        with tc.tile_pool(name="p3kv", bufs=2) as p3kv, \
             tc.tile_pool(name="p3", bufs=3) as p3, \
             tc.tile_pool(name="p3e", bufs=4) as p3e, \
             tc.tile_pool(name="p3ps", bufs=2, space="PSUM") as p3ps, \
             tc.tile_pool(name="p3po", bufs=1, space="PSUM") as p3po:
            for (t0, L) in SEQS:
                ntil = L // 128
                for h in range(HPC):
                    hs = slice(h * 128, (h + 1) * 128)
                    q_cm = p3kv.tile([128, 1536], F16, tag="q_cm")
                    k_cm = p3kv.tile([128, 1536], F16, tag="k_cm")
                    vt = p3kv.tile([128, 12, 128], F16, tag="vt")
                    nc.sync.dma_start(
                        out=vt[:, :ntil, :],
                        in_=v_tm[t0:t0 + L, hs].rearrange(
                            "(c p) d -> p c d", p=128))
                    for i in range(ntil):
                        for (src, dst) in ((q_tm, q_cm), (k_tm, k_cm)):
                            lt = p3.tile([128, 128], F16, tag="lt")
                            nc.sync.dma_start(
                                out=lt[:],
                                in_=src[t0 + i * 128:t0 + (i + 1) * 128, hs])
                            pt = p3ps.tile([128, 128], F16, tag="pt3")
                            nc.tensor.transpose(pt[:], lt[:], ident16[:])
                            nc.scalar.copy(dst[:, i * 128:(i + 1) * 128],
                                           pt[:])
                    # Lq chunks
                    qcs = []
                    qs = 0
                    while qs < L:
                        qn = min(512, L - qs)
                        qcs.append((qs, qn))
                        qs += qn
                    for qs, qn in qcs:
                        nlk = (qs + qn) // 128
                        mb0 = qs // 128
                        ps_o = p3po.tile([128, 512], F32, tag="ps_o")
                        ps_sum = p3po.tile([1, 512], F32, tag="ps_sum")
                        for i in range(nlk):
                            ps_s = p3ps.tile([128, 512], F32, tag="ps_s")
                            nc.tensor.matmul(
                                ps_s[:, :qn], k_cm[:, i * 128:(i + 1) * 128],
                                q_cm[:, qs:qs + qn], start=True, stop=True)
                            e = p3e.tile([128, 512], F16, tag="e")
                            nc.scalar.activation(e[:, :qn], ps_s[:, :qn],
                                                 AF.Exp, bias=0.0,
                                                 scale=SCALE)
                            if i >= mb0:
                                m = i - mb0
                                nc.vector.tensor_tensor(
                                    out=e[:, :qn], in0=e[:, :qn],
                                    in1=dm[:, m, :qn], op=OP.mult)
                            nc.tensor.matmul(
                                ps_sum[:, :qn], ones16[:], e[:, :qn],
                                start=(i == 0), stop=(i == nlk - 1))
                            nc.tensor.matmul(
                                ps_o[:, :qn], vt[:, i, :], e[:, :qn],
                                start=(i == 0), stop=(i == nlk - 1))
                        ssum = p3.tile([1, 512], F32, tag="ssum")
                        nc.scalar.copy(ssum[:, :qn], ps_sum[:, :qn])
                        rsum = p3.tile([1, 512], F32, tag="rsum")
                        nc.vector.reciprocal(rsum[:, :qn], ssum[:, :qn])
                        ps_b = p3ps.tile([128, 512], F32, tag="ps_s")
                        nc.tensor.matmul(ps_b[:, :qn], ones32[:],
                                         rsum[:, :qn], start=True, stop=True)
                        bb = p3.tile([128, 512], F32, tag="bb")
                        nc.scalar.copy(bb[:, :qn], ps_b[:, :qn])
                        of = p3.tile([128, 512], F16, tag="of")
                        nc.vector.tensor_tensor(out=of[:, :qn],
                                                in0=ps_o[:, :qn],
                                                in1=bb[:, :qn], op=OP.mult)
                        for (tbi, ti, off, ln) in _tok_runs(t0 + qs, qn):
                            nc.sync.dma_start(
                                out=a2a_in[tbi, hs, ti:ti + ln],
                                in_=of[:, off:off + ln])

            # ------------------------------------------------- P3b: decode
            for h in range(HPC):
                hs = slice(h * 128, (h + 1) * 128)
                qd = p3.tile([128, 16], F16, tag="qd")
                kd = p3.tile([128, 16], F16, tag="kd")
                for (src, dst) in ((q_tm, qd), (k_tm, kd)):
                    lt = p3.tile([128, 128], F16, tag="lt")
                    nc.sync.dma_start(out=lt[:16], in_=src[DOFF:T, hs])
                    pt = p3ps.tile([128, 128], F16, tag="pt3")
                    nc.tensor.transpose(pt[:, :16], lt[:16],
                                        ident16[:16, :16])
                    nc.scalar.copy(dst[:], pt[:, :16])

                esa = p3.tile([1, 16], F32, tag="esa")
                esb = p3.tile([1, 16], F32, tag="esb")
                od = p3.tile([16, 128], F32, tag="od")
                for b in range(DEC_B):
                    kc = p3e.tile([128, PAST], F16, tag="kc")
                    nc.sync.dma_start(out=kc[:], in_=kct[h, b])
                    vc = p3e.tile([128, 4, 128], F16, tag="vc")
                    nc.sync.dma_start(
                        out=vc[:],
                        in_=vct[b, h].rearrange("(c p) d -> p c d", p=128))
                    ps1 = p3ps.tile([1, 512], F32, tag="ps_s")
                    nc.tensor.matmul(ps1[:], qd[:, b:b + 1], kc[:],
                                     start=True, stop=True)
                    psn = p3ps.tile([1, 1], F32, tag="pt3")
                    nc.tensor.matmul(psn[:], qd[:, b:b + 1], kd[:, b:b + 1],
                                     start=True, stop=True)
                    e1 = p3e.tile([1, 512], F16, tag="e1")
                    nc.scalar.activation(e1[:], ps1[:], AF.Exp, bias=0.0,
                                         scale=SCALE,
                                         accum_out=esa[:, b:b + 1])
                    en = p3e.tile([1, 1], F16, tag="en")
                    nc.scalar.activation(en[:], psn[:], AF.Exp, bias=0.0,
                                         scale=SCALE,
                                         accum_out=esb[:, b:b + 1])
                    psT = p3ps.tile([128, 4], F32, tag="pt3")
                    for c in range(4):
                        nc.tensor.matmul(
                            psT[:, c:c + 1], e1[:, c * 128:(c + 1) * 128],
                            ones16[:1, :1], start=True, stop=True)
                    eT = p3e.tile([128, 4], F16, tag="eT")
                    nc.vector.tensor_copy(eT[:], psT[:])
                    ps_ob = p3ps.tile([1, 128], F32, tag="pt3")
                    for c in range(4):
                        nc.tensor.matmul(ps_ob[:], eT[:, c:c + 1],
                                         vc[:, c, :], start=(c == 0),
                                         stop=False)
                    vd1 = p3e.tile([1, 128], F16, tag="vd1")
                    nc.sync.dma_start(out=vd1[:],
                                      in_=v_tm[DOFF + b:DOFF + b + 1, hs])
                    nc.tensor.matmul(ps_ob[:], en[:1, :1], vd1[:],
                                     start=False, stop=True)
                    nc.scalar.copy(od[b:b + 1, :], ps_ob[:])
                esum = p3.tile([1, 16], F32, tag="esum")
                nc.vector.tensor_tensor(out=esum[:], in0=esa[:], in1=esb[:],
                                        op=OP.add)
                rsd = p3.tile([1, 16], F32, tag="rsd")
                nc.vector.reciprocal(rsd[:], esum[:])
                oT = p3ps.tile([128, 16], F32, tag="ps_s")
                nc.tensor.matmul(oT[:], od[:], ident32[:16, :16],
                                 start=True, stop=True)
                ps_bd = p3ps.tile([128, 16], F32, tag="pt3")
                nc.tensor.matmul(ps_bd[:], ones32[:], rsd[:],
                                 start=True, stop=True)
                bbd = p3.tile([128, 16], F32, tag="bbd")
                nc.scalar.copy(bbd[:], ps_bd[:])
                ofd = p3.tile([128, 16], F16, tag="ofd")
                nc.vector.tensor_tensor(out=ofd[:], in0=oT[:], in1=bbd[:],
                                        op=OP.mult)
                nc.sync.dma_start(out=a2a_in[NCORES - 1, hs, TB - DEC_B:TB],
                                  in_=ofd[:])

        # ================================================ P4: a2a + o_proj
        with tc.tile_pool(name="p4", bufs=2) as p4, \
             tc.tile_pool(name="p4rq", bufs=1) as p4rq, \
             tc.tile_pool(name="p4w", bufs=8) as p4w, \
             tc.tile_pool(name="p4ps", bufs=2, space="PSUM") as p4ps, \
             tc.tile_pool(name="p4po", bufs=2, space="PSUM") as p4po:
            nc.gpsimd.collective_compute(
                "AllToAll", mybir.AluOpType.bypass,
                replica_groups=[rg],
                ins=[a2a_in[:]], outs=[a2a_out[:]])

            rqall = p4rq.tile([128, NG, TB], F16)
            tchunks = [(o, min(128, TB - o)) for o in range(0, TB, 128)]
            for g in range(NG):
                maxq = 7.0 if g < NG - 1 else 127.0
                rgt = p4.tile([128, TB], F16, tag="rgt")
                nc.gpsimd.indirect_dma_start(
                    out=rgt[:], out_offset=None, in_=a2a_out[:],
                    in_offset=bass.IndirectOffsetOnAxis(
                        ap=rixt[:, g:g + 1], axis=0))
                st = p4.tile([128, 4], F32, tag="st")
                for c, (o, w) in enumerate(tchunks):
                    ptc = p4ps.tile([128, 128], F16, tag="ptc")
                    nc.tensor.transpose(ptc[:w, :], rgt[:, o:o + w],
                                        ident16[:])
                    tcs = p4.tile([128, 128], F16, tag="tcs")
                    nc.scalar.copy(tcs[:w], ptc[:w])
                    nc.vector.tensor_reduce(
                        out=st[:w, c:c + 1], in_=tcs[:w],
                        axis=AX.X, op=OP.max, apply_absolute_value=True)
                sc4 = p4.tile([128, 4], F32, tag="sc4")
                nc.vector.tensor_scalar(
                    out=sc4[:], in0=st[:], scalar1=1.0 / maxq,
                    scalar2=1e-6, op0=OP.mult, op1=OP.add)
                rv4 = p4.tile([128, 4], F32, tag="rv4")
                nc.vector.reciprocal(rv4[:], sc4[:])
                # flatten per-chunk scale columns into [1, TB] rows via PE
                ps_srow = p4ps.tile([1, 512], F32, tag="psrow")
                ps_rrow = p4ps.tile([1, 512], F32, tag="psrow")
                for c, (o, w) in enumerate(tchunks):
                    nc.tensor.matmul(ps_srow[:, o:o + w], sc4[:w, c:c + 1],
                                     ident32[:w, :w], start=True, stop=True)
                    nc.tensor.matmul(ps_rrow[:, o:o + w], rv4[:w, c:c + 1],
                                     ident32[:w, :w], start=True, stop=True)
                srow = p4.tile([1, 512], F32, tag="srow")
                nc.scalar.copy(srow[:, :TB], ps_srow[:, :TB])
                rrow = p4.tile([1, 512], F32, tag="rrow")
                nc.scalar.copy(rrow[:, :TB], ps_rrow[:, :TB])
                ps_sb = p4ps.tile([128, 512], F32, tag="psbc")
                nc.tensor.matmul(ps_sb[:, :TB], ones32[:], srow[:, :TB],
                                 start=True, stop=True)
                ps_rb = p4ps.tile([128, 512], F32, tag="psbc")
                nc.tensor.matmul(ps_rb[:, :TB], ones32[:], rrow[:, :TB],
                                 start=True, stop=True)
                q1 = p4.tile([128, TB], F32, tag="q1")
                nc.vector.tensor_tensor(out=q1[:], in0=ps_rb[:, :TB],
                                        in1=rgt[:], op=OP.mult)
                qi4 = p4.tile([128, TB], I32, tag="qi4")
                nc.vector.tensor_copy(qi4[:], q1[:])
                qf4 = p4.tile([128, TB], F32, tag="qf4")
                nc.vector.tensor_copy(qf4[:], qi4[:])
                nc.vector.tensor_tensor(out=rqall[:, g, :],
                                        in0=ps_sb[:, :TB],
                                        in1=qf4[:], op=OP.mult)

            for ocb in range(NG):
                ps_out = p4po.tile([128, TB], F32, tag="ps_out")
                for icb in range(NG):
                    wt = p4w.tile([128, 128], F16, tag="wt")
                    nc.sync.dma_start(
                        out=wt[:],
                        in_=wo_t[icb * 128:(icb + 1) * 128,
                                 ocb * 128:(ocb + 1) * 128])
                    nc.tensor.matmul(ps_out[:], wt[:], rqall[:, icb, :],
                                     start=(icb == 0), stop=(icb == NG - 1))
                outf = p4.tile([128, TB], F32, tag="outf")
                nc.scalar.copy(outf[:], ps_out[:])
                nc.sync.dma_start(out=out[ocb * 128:(ocb + 1) * 128, :],
                                  in_=outf[:])

        res.release()
    return nc


# ==========================================================================
# device runner (modeled on bass2jax.run_bass_via_pjrt, with repeat timing)
# ==========================================================================
def _run_device(in_maps):
    global LAST_HW_EXEC_NS
    import jax
    import numpy as _np
    from jax.sharding import Mesh, PartitionSpec
    from jax.experimental.shard_map import shard_map
    from concourse import bass2jax, mybir

    nc = _build_program()
    bass2jax.install_neuronx_cc_hook()

    in_names, out_names, out_avals, zero_outs = [], [], [], []
    for alloc in nc.m.functions[0].allocations:
        if not isinstance(alloc, mybir.MemoryLocationSet):
            continue
        name = alloc.memorylocations[0].name
        if alloc.kind == "ExternalInput":
            if nc.partition_id_tensor is None or \
                    name != nc.partition_id_tensor.name:
                in_names.append(name)
        elif alloc.kind == "ExternalOutput":
            out_names.append(name)
            shape = tuple(alloc.tensor_shape)
            dtype = mybir.dt.np(alloc.dtype)
            out_avals.append(jax.core.ShapedArray(shape, dtype))
            zero_outs.append(_np.zeros(shape, dtype))
    n_params = len(in_names)
    n_outs = len(out_avals)
    all_names = list(in_names) + list(out_names)
    if nc.partition_id_tensor is not None:
        all_names.append(nc.partition_id_tensor.name)

    from concourse.bass2jax import _bass_exec_p, partition_id_tensor

    def _body(*args):
        operands = list(args)
        if nc.partition_id_tensor is not None:
            operands.append(partition_id_tensor())
        outs = _bass_exec_p.bind(
            *operands,
            out_avals=tuple(out_avals),
            in_names=tuple(all_names),
            out_names=tuple(out_names),
            lowering_input_output_aliases=(),
            sim_require_finite=True,
            sim_require_nnan=True,
            nc=nc,
        )
        return tuple(outs)

    devices = jax.devices()[:NCORES]
    mesh = Mesh(np.asarray(devices), ("core",))
    in_specs = (PartitionSpec("core"),) * (n_params + n_outs)
    out_specs = (PartitionSpec("core"),) * n_outs
    sharded = jax.jit(shard_map(_body, mesh=mesh, in_specs=in_specs,
                                out_specs=out_specs, check_rep=False),
                      keep_unused=True)
    concat_in = [
        np.concatenate([np.asarray(in_maps[c][nm]) for c in range(NCORES)],
                       axis=0)
        for nm in in_names[:n_params]
    ]
    concat_zeros = [
        np.zeros((NCORES * z.shape[0], *z.shape[1:]), z.dtype)
        for z in zero_outs
    ]
    args = [jax.device_put(a) for a in concat_in + concat_zeros]
    out_arrs = sharded(*args)
    jax.block_until_ready(out_arrs)
    # timed repeats (NEFF compiled & cached now)
    best = None
    for _ in range(3):
        t0 = time.perf_counter()
        o2 = sharded(*args)
        jax.block_until_ready(o2)
        dt = time.perf_counter() - t0
        best = dt if best is None or dt < best else best
    LAST_HW_EXEC_NS = int(best * 1e9)

    results = []
    for c in range(NCORES):
        results.append({
            nm: np.asarray(out_arrs[i]).reshape(NCORES, *out_avals[i].shape)[c]
            for i, nm in enumerate(out_names)
        })
    return results


# ==========================================================================
# exact numpy fallback
# ==========================================================================
def _qdq(x, bits, group):
    maxq = 2.0 ** (bits - 1) - 1.0
    xg = x.reshape(x.shape[:-1] + (-1, group))
    s = np.max(np.abs(xg), axis=-1, keepdims=True) / maxq + 1e-6
    q = (np.clip(np.round(xg / s), -maxq - 1.0, maxq) * s).reshape(x.shape)
    return q.astype(np.float32)


def _rope_np(x, beg):
    d, L = x.shape[-1], x.shape[-2]
    inv = 1.0 / (10000.0 ** (np.arange(0, d, 2, dtype=np.float32) / d))
    t = np.arange(L, dtype=np.float32) + beg
    emb = np.concatenate([t[:, None] * inv[None, :]] * 2, axis=-1)
    rh = np.concatenate([-x[..., d // 2:], x[..., : d // 2]], axis=-1)
    return (x * np.cos(emb) + rh * np.sin(emb)).astype(np.float32)


def _softmax(s, axis):
    m = np.max(s, axis=axis, keepdims=True)
    e = np.exp(s - m)
    return e / np.sum(e, axis=axis, keepdims=True)


def _run_numpy(hidden_states, wq, wk, wv, wo, ridx, cache_k, cache_v):
    hq = np.concatenate(
        [_qdq(hidden_states[:, : H - N_OUTLIER], 4, GROUP),
         _qdq(hidden_states[:, H - N_OUTLIER:], 8, N_OUTLIER)], axis=-1)
    q = hq @ wq.T
    k = hq @ wk.T
    v = hq @ wv.T
    k = _qdq(k.reshape(-1, NH, HD), 4, KV_GROUP).reshape(-1, H)
    v = _qdq(v.reshape(-1, NH, HD), 4, KV_GROUP).reshape(-1, H)
    outs = []
    off = 0
    for L in PREFILL:
        qs = _rope_np(q[off:off + L].reshape(L, NH, HD).transpose(1, 0, 2), 0)
        kk = _rope_np(k[off:off + L].reshape(L, NH, HD).transpose(1, 0, 2), 0)
        vv = v[off:off + L].reshape(L, NH, HD).transpose(1, 0, 2)
        sc = np.einsum('hqd,hkd->hqk', qs, kk) * SCALE
        mask = np.tril(np.ones((L, L), dtype=bool))
        sc = np.where(mask, sc, -np.inf)
        a = _softmax(sc, axis=-1)
        o = np.einsum('hqk,hkd->hqd', a, vv).transpose(1, 0, 2)
        outs.append(o.reshape(L, H))
        off += L
    qd = _rope_np(q[DOFF:].reshape(DEC_B, NH, 1, HD), PAST)[:, :, 0]
    kd = _rope_np(k[DOFF:].reshape(DEC_B, NH, 1, HD), PAST)
    vd = v[DOFF:].reshape(DEC_B, NH, 1, HD)
    K = np.concatenate([cache_k, kd], axis=2)
    V = np.concatenate([cache_v, vd], axis=2)
    s = np.einsum('bhd,bhsd->bhs', qd, K) * SCALE
    a = _softmax(s, axis=-1)
    outs.append(np.einsum('bhs,bhsd->bhd', a, V).reshape(DEC_B, H))
    attn = np.concatenate(outs, axis=0)
    reordered = attn[:, ridx]
    ro = np.concatenate(
        [_qdq(reordered[:, : H - N_OUTLIER], 4, GROUP),
         _qdq(reordered[:, H - N_OUTLIER:], 8, N_OUTLIER)], axis=-1)
    return (ro @ wo.T).astype(np.float32)


# ==========================================================================
def kernel(hidden_states, wq, wk, wv, wo, reorder_index, cache_k, cache_v):
    hidden_states = np.asarray(hidden_states, np.float32)
    wq = np.asarray(wq, np.float32)
    wk = np.asarray(wk, np.float32)
    wv = np.asarray(wv, np.float32)
    wo = np.asarray(wo, np.float32)
    ridx = np.asarray(reorder_index).astype(np.int64)
    cache_k = np.asarray(cache_k, np.float32)
    cache_v = np.asarray(cache_v, np.float32)

    if not os.environ.get("KERNEL_FORCE_NUMPY"):
        try:
            in_maps = _host_prep(hidden_states, wq, wk, wv, wo, ridx,
                                 cache_k, cache_v)
            results = _run_device(in_maps)
            full = np.empty((T, H), dtype=np.float32)
            for r in range(NCORES):
                full[TB * r:TB * (r + 1), :] = results[r]["out"].T
            return full
        except Exception:
            traceback.print_exc()
            print("kernel: device path failed; numpy fallback",
                  file=sys.stderr)
    return _run_numpy(hidden_states, wq, wk, wv, wo, ridx, cache_k, cache_v)
